# revision 36
# baseline (speedup 1.0000x reference)
"""Global-KNN GCN kernel for Trainium2 (8 NeuronCores, SPMD).

Heavy part (161 GFLOP pairwise-score matmul + per-chunk top-8) runs on
device, row-sharded 784 rows/core. Scores s_ij = x_i.x_j - 0.5*||x_j||^2
rank identically to -squared-distance. The pairwise matmul runs in
fp8e4m3 with perf_mode=DoubleRow (K=256 per instruction, 2x PE rate,
4x less HBM traffic than fp32); the -0.5||x_j||^2 bias is residual-coded
into three stolen feature slots (2045..2047) so it rides inside the last
contraction chunk for free. Top-8 per 448-column chunk via DVE max8 +
max_index reading PSUM directly (14x8 = 112 candidates per row; the
true top-9 is among them unless 9+ of them land in one chunk, P~1e-8).
fp8 score noise (std ~1.7, validated 0 misses) is absorbed by an exact
fp32 re-score of the best 32 candidates on host. Host also does the
cheap part: edge list, sym norm, two sparse aggregations and the two
small dense layers.
"""

import os
import sys
import numpy as np
import ml_dtypes

try:
    import concourse  # noqa: F401
except ImportError:  # harness may not have the bass repo on sys.path
    sys.path.insert(0, "/opt/trn_rl_repo")

B, H, W, C = 32, 14, 14, 2048
N = B * H * W            # 6272 nodes
K = 8                    # neighbors (excluding self)
N_CORES = 8
ROWS = N // N_CORES      # 784 rows per core
MT, MP = 7, 112          # 7 partition tiles of 112 rows = 784
NB = 448                 # psum tile free size (one bank; 6272 = 14*448)
NJ = N // NB             # 14 column chunks
KP = C // 256            # 8 double-row contraction chunks

LAST_EXEC_NS = None
LAST_KNN = None
_PROG = None


def _build_program():
    from concourse import bacc, tile, mybir

    f32 = mybir.dt.float32
    f8 = mybir.dt.float8e4
    u16 = mybir.dt.uint16
    DR = mybir.MatmulPerfMode.DoubleRow

    nc = bacc.Bacc("TRN2", target_bir_lowering=False)
    # [p, j, kp, s, c] = x8[col j*448+c, feat kp*256+s*128+p]
    # (features 2045..2047 carry the -0.5||x_j||^2 bias, fp8-residual-coded)
    rhs8 = nc.declare_dram_parameter("rhs8", [128, NJ, KP, 2, NB], f8, isOutput=False)
    # [p, t*8+kp, s, m] = x8[own row t*112+m, feat kp*256+s*128+p]
    # (features 2045..2047 hold the bias decode weights 64, 8, 1)
    lhsT8 = nc.declare_dram_parameter("lhsT8", [128, MT * KP, 2, MP], f8, isOutput=False)
    vals = nc.declare_dram_parameter("vals", [MT, MP, NJ, 8], f32, isOutput=True)
    idxs = nc.declare_dram_parameter("idxs", [MT, MP, NJ, 8], u16, isOutput=True)

    with tile.TileContext(nc) as tc:
        with (
            tc.tile_pool(name="persist", bufs=1) as pp,
            tc.tile_pool(name="rhs", bufs=4) as rp,
            tc.tile_pool(name="psum", bufs=8, space="PSUM") as psp,
        ):
            lhsT = pp.tile([128, MT * KP, 2, MP], f8)
            vst = [pp.tile([MP, NJ, 8], f32, name=f"vst_{t}") for t in range(MT)]
            ist = [pp.tile([MP, NJ, 8], u16, name=f"ist_{t}") for t in range(MT)]

            rhs_t = {}

            def load_rhs(j, split=2):
                r = rp.tile([128, KP, 2, NB], f8, tag="rhs", name=f"rhs_{j}")
                step = KP // split
                for k0 in range(0, KP, step):
                    nc.sync.dma_start(
                        out=r[:, k0:k0 + step, :, :],
                        in_=rhs8[:, j, k0:k0 + step, :, :],
                    )
                rhs_t[j] = r

            def load_lhsT(t):
                # issue from the (otherwise idle) scalar engine: lands on the
                # second HW-DGE ring, so it never queues ahead of the
                # time-critical rhs stream on the sync ring
                nc.scalar.dma_start(
                    out=lhsT[:, t * KP:(t + 1) * KP, :, :],
                    in_=lhsT8[:, t * KP:(t + 1) * KP, :, :],
                )

            # DMA issue order matters (sync-engine FIFO): row-tile 0's
            # weights, then the first rhs chunk in fine-grained slices (so
            # the first matmuls start after 1/8 of it arrives), then the
            # rest interleaved.
            load_lhsT(0)
            load_rhs(0, split=8)
            load_lhsT(1)
            load_rhs(1, split=4)
            for t in range(2, MT):
                load_lhsT(t)

            # warm-up: ONE long accumulation group of small DoubleRow
            # matmuls on row-tile 0's (already arriving) weight panels keeps
            # the PE busy through the HAM activity window (~3.4us), so the
            # real matmul stream starts at 2.4 GHz instead of 1.2 GHz.
            # (Separate start/stop groups serialize ~450ns apart and never
            # warm the clock — measured; a single group pipelines.)
            wps = psp.tile([MP, MP], f32, tag="ps", name="ps_warm")
            NWU = 35
            for i in range(NWU):
                nc.tensor.matmul(
                    wps[:], lhsT[:, 0, :, :], lhsT[:, i % KP, :, :],
                    start=(i == 0), stop=(i == NWU - 1), perf_mode=DR,
                    skip_group_check=True,
                )

            for j in range(NJ):                 # 14 column chunks
                rhs = rhs_t.pop(j)
                for t in range(MT):
                    ps = psp.tile([MP, NB], f32, tag="ps", name=f"ps_{j}_{t}")
                    for ki in range(KP):
                        nc.tensor.matmul(
                            ps[:], lhsT[:, t * KP + ki, :, :], rhs[:, ki, :, :],
                            start=(ki == 0), stop=(ki == KP - 1), perf_mode=DR,
                            skip_group_check=True,
                        )
                    nc.vector.max(vst[t][:, j, :], ps[:])
                    nc.vector.max_index(ist[t][:, j, :], vst[t][:, j, :], ps[:])
                    if t == 0 and j + 2 < NJ:
                        load_rhs(j + 2)
            for t in range(MT):
                nc.scalar.dma_start(out=vals[t, :, :, :], in_=vst[t][:, :, :])
                nc.scalar.dma_start(out=idxs[t, :, :, :], in_=ist[t][:, :, :])
    nc.compile()
    return nc


def _knn_from_device(x_flat):
    """Run the SPMD program; return knn [N, K] int64 global indices."""
    global LAST_EXEC_NS, _PROG
    from concourse.bass_utils import run_bass_kernel_spmd

    if _PROG is None:
        _PROG = _build_program()

    f8 = ml_dtypes.float8_e4m3
    xq = x_flat.astype(f8)                                   # [N, C]
    # encode -0.5||x_j||^2 into feature slots 2045..2047 (fp8 residual code);
    # the row side holds the decode weights 64, 8, 1 there instead.
    nh = -0.5 * np.sum(x_flat * x_flat, axis=1, dtype=np.float32)
    v1 = (nh / 64).astype(f8)
    r1 = nh - 64 * v1.astype(np.float32)
    v2 = (r1 / 8).astype(f8)
    v3 = (r1 - 8 * v2.astype(np.float32)).astype(f8)
    rhsq = xq.copy()
    rhsq[:, 2045], rhsq[:, 2046], rhsq[:, 2047] = v1, v2, v3
    lhsq = xq
    lhsq[:, 2045], lhsq[:, 2046], lhsq[:, 2047] = 64, 8, 1
    rhsT = np.ascontiguousarray(rhsq.T)                      # [C, N]
    lhsT = np.ascontiguousarray(lhsq.T)
    rhs8 = np.ascontiguousarray(
        rhsT.reshape(KP, 2, 128, NJ, NB).transpose(2, 3, 0, 1, 4))
    in_maps = []
    for c in range(N_CORES):
        xrqT = lhsT[:, c * ROWS:(c + 1) * ROWS]              # [C, 784]
        lhsT8 = np.ascontiguousarray(
            xrqT.reshape(KP, 2, 128, MT, MP).transpose(2, 3, 0, 1, 4)
        ).reshape(128, MT * KP, 2, MP)
        in_maps.append({"rhs8": rhs8, "lhsT8": lhsT8})
    res = run_bass_kernel_spmd(
        _PROG, in_maps, list(range(N_CORES)),
        trace=bool(os.environ.get("KNN_TRACE")),
    )
    if res.exec_time_ns is not None:
        LAST_EXEC_NS = res.exec_time_ns

    # per-core outputs are [MT, MP, NJ, 8] -> [ROWS, 112]
    vals_all = np.concatenate(
        [r["vals"].reshape(ROWS, 112) for r in res.results], axis=0)
    loc = np.concatenate(
        [r["idxs"].reshape(ROWS, 112) for r in res.results],
        axis=0).astype(np.int64)
    idxs_all = loc + (np.arange(NJ, dtype=np.int64) * NB).repeat(8)[None, :]

    # coarse top-32 by device (fp8 matmul) score, then exact fp32 re-score
    part = np.argpartition(-vals_all, 32, axis=1)[:, :32]
    idxs_all = np.take_along_axis(idxs_all, part, axis=1)    # [N, 32]
    sq = np.sum(x_flat * x_flat, axis=1, dtype=np.float32)
    exact = np.empty((N, 32), dtype=np.float32)
    BLK = 196
    for r0 in range(0, N, BLK):
        r1 = r0 + BLK
        cand = idxs_all[r0:r1]                               # [b, 32]
        xc = x_flat[cand]                                    # [b, 32, C]
        exact[r0:r1] = np.einsum("bc,bkc->bk", x_flat[r0:r1], xc,
                                 dtype=np.float32) - 0.5 * sq[cand]
    order = np.argsort(-exact, axis=1, kind="stable")[:, :K + 1]
    top = np.take_along_axis(idxs_all, order, axis=1)        # [N, 9]
    rows = np.arange(N)[:, None]
    selfpos = top == rows
    has_self = selfpos.any(axis=1)
    rem = np.where(has_self, selfpos.argmax(axis=1), K)      # drop self, else 9th
    keep = np.ones((N, K + 1), dtype=bool)
    keep[np.arange(N), rem] = False
    global LAST_KNN
    LAST_KNN = top[keep].reshape(N, K)
    return LAST_KNN


def kernel(x, W1, b1, W2, b2):
    x = np.asarray(x, dtype=np.float32)
    W1 = np.asarray(W1, dtype=np.float32)
    b1 = np.asarray(b1, dtype=np.float32)
    W2 = np.asarray(W2, dtype=np.float32)
    b2 = np.asarray(b2, dtype=np.float32)

    xf = x.reshape(N, C)
    knn = _knn_from_device(xf)

    src = np.repeat(np.arange(N, dtype=np.int64), K)
    dst = knn.reshape(-1)
    loops = np.arange(N, dtype=np.int64)
    src = np.concatenate([src, loops])
    dst = np.concatenate([dst, loops])

    deg = np.bincount(dst, minlength=N).astype(np.float32)
    dinv = 1.0 / np.sqrt(np.maximum(deg, 1.0))
    norm = (dinv[src] * dinv[dst]).astype(np.float32)

    try:
        import scipy.sparse as sps
        A = sps.csr_matrix((norm, (dst, src)), shape=(N, N), dtype=np.float32)

        def agg(hw):
            return A @ hw
    except Exception:
        def agg(hw):
            out = np.zeros_like(hw)
            np.add.at(out, dst, hw[src] * norm[:, None])
            return out

    h1 = np.maximum(agg(xf @ W1) + b1, 0.0).astype(np.float32)
    h2 = np.maximum(agg(h1 @ W2) + b2, 0.0).astype(np.float32)
    return h2.reshape(B, H, W, W2.shape[1]).astype(np.float32)


# revision 39
# speedup vs baseline: 1.0134x; 1.0134x over previous
"""Global-KNN GCN kernel for Trainium2 (8 NeuronCores, SPMD).

Heavy part (161 GFLOP pairwise-score matmul + per-chunk top-8) runs on
device, row-sharded 784 rows/core. Scores s_ij = x_i.x_j - 0.5*||x_j||^2
rank identically to -squared-distance. The pairwise matmul runs in
fp8e4m3 with perf_mode=DoubleRow (K=256 per instruction, 2x PE rate,
4x less HBM traffic than fp32); the -0.5||x_j||^2 bias is residual-coded
into three stolen feature slots (2045..2047) so it rides inside the last
contraction chunk for free. Top-8 per 448-column chunk via DVE max8 +
max_index reading PSUM directly (14x8 = 112 candidates per row; the
true top-9 is among them unless 9+ of them land in one chunk, P~1e-8).
fp8 score noise (std ~1.7, validated 0 misses) is absorbed by an exact
fp32 re-score of the best 32 candidates on host. Host also does the
cheap part: edge list, sym norm, two sparse aggregations and the two
small dense layers.
"""

import os
import sys
import numpy as np
import ml_dtypes

try:
    import concourse  # noqa: F401
except ImportError:  # harness may not have the bass repo on sys.path
    sys.path.insert(0, "/opt/trn_rl_repo")

B, H, W, C = 32, 14, 14, 2048
N = B * H * W            # 6272 nodes
K = 8                    # neighbors (excluding self)
N_CORES = 8
ROWS = N // N_CORES      # 784 rows per core
MT, MP = 7, 112          # 7 partition tiles of 112 rows = 784
NB = 448                 # psum tile free size (one bank; 6272 = 14*448)
NJ = N // NB             # 14 column chunks
KP = C // 256            # 8 double-row contraction chunks

LAST_EXEC_NS = None
LAST_KNN = None
_PROG = None


def _build_program():
    from concourse import bacc, tile, mybir

    f32 = mybir.dt.float32
    f8 = mybir.dt.float8e4
    u16 = mybir.dt.uint16
    DR = mybir.MatmulPerfMode.DoubleRow

    nc = bacc.Bacc("TRN2", target_bir_lowering=False)
    # [p, j, kp, s, c] = x8[col j*448+c, feat kp*256+s*128+p]
    # (features 2045..2047 carry the -0.5||x_j||^2 bias, fp8-residual-coded)
    rhs8 = nc.declare_dram_parameter("rhs8", [128, NJ, KP, 2, NB], f8, isOutput=False)
    # [p, t*8+kp, s, m] = x8[own row t*112+m, feat kp*256+s*128+p]
    # (features 2045..2047 hold the bias decode weights 64, 8, 1)
    lhsT8 = nc.declare_dram_parameter("lhsT8", [128, MT * KP, 2, MP], f8, isOutput=False)
    vals = nc.declare_dram_parameter("vals", [MT, MP, NJ, 8], f32, isOutput=True)
    idxs = nc.declare_dram_parameter("idxs", [MT, MP, NJ, 8], u16, isOutput=True)

    with tile.TileContext(nc) as tc:
        with (
            tc.tile_pool(name="persist", bufs=1) as pp,
            tc.tile_pool(name="rhs", bufs=6) as rp,
            tc.tile_pool(name="psum", bufs=8, space="PSUM") as psp,
        ):
            lhsT = pp.tile([128, MT * KP, 2, MP], f8)
            vst = [pp.tile([MP, NJ, 8], f32, name=f"vst_{t}") for t in range(MT)]
            ist = [pp.tile([MP, NJ, 8], u16, name=f"ist_{t}") for t in range(MT)]

            rhs_t = {}

            def load_rhs(j, split=2):
                r = rp.tile([128, KP, 2, NB], f8, tag="rhs", name=f"rhs_{j}")
                step = KP // split
                for k0 in range(0, KP, step):
                    nc.sync.dma_start(
                        out=r[:, k0:k0 + step, :, :],
                        in_=rhs8[:, j, k0:k0 + step, :, :],
                    )
                rhs_t[j] = r

            def load_lhsT(t):
                # issue from the (otherwise idle) scalar engine: lands on the
                # second HW-DGE ring, so it never queues ahead of the
                # time-critical rhs stream on the sync ring
                nc.scalar.dma_start(
                    out=lhsT[:, t * KP:(t + 1) * KP, :, :],
                    in_=lhsT8[:, t * KP:(t + 1) * KP, :, :],
                )

            # DMA issue order matters (sync-engine FIFO): row-tile 0's
            # weights, then the first rhs chunk in fine-grained slices (so
            # the first matmuls start after 1/8 of it arrives), then the
            # rest interleaved.
            load_lhsT(0)
            load_rhs(0, split=8)
            load_lhsT(1)
            load_rhs(1, split=4)
            for t in range(2, MT):
                load_lhsT(t)

            for j in range(NJ):                 # 14 column chunks
                rhs = rhs_t.pop(j)
                for t in range(MT):
                    ps = psp.tile([MP, NB], f32, tag="ps", name=f"ps_{j}_{t}")
                    for ki in range(KP):
                        nc.tensor.matmul(
                            ps[:], lhsT[:, t * KP + ki, :, :], rhs[:, ki, :, :],
                            start=(ki == 0), stop=(ki == KP - 1), perf_mode=DR,
                            skip_group_check=True,
                        )
                    nc.vector.max(vst[t][:, j, :], ps[:])
                    nc.vector.max_index(ist[t][:, j, :], vst[t][:, j, :], ps[:])
                    if t == 0 and j + 2 < NJ and j + 2 not in rhs_t:
                        load_rhs(j + 2)
                    if t == 3 and j + 3 < NJ and j + 3 not in rhs_t:
                        load_rhs(j + 3)
            for t in range(MT):
                nc.scalar.dma_start(out=vals[t, :, :, :], in_=vst[t][:, :, :])
                nc.scalar.dma_start(out=idxs[t, :, :, :], in_=ist[t][:, :, :])
    nc.compile()
    return nc


def _knn_from_device(x_flat):
    """Run the SPMD program; return knn [N, K] int64 global indices."""
    global LAST_EXEC_NS, _PROG
    from concourse.bass_utils import run_bass_kernel_spmd

    if _PROG is None:
        _PROG = _build_program()

    f8 = ml_dtypes.float8_e4m3
    xq = x_flat.astype(f8)                                   # [N, C]
    # encode -0.5||x_j||^2 into feature slots 2045..2047 (fp8 residual code);
    # the row side holds the decode weights 64, 8, 1 there instead.
    nh = -0.5 * np.sum(x_flat * x_flat, axis=1, dtype=np.float32)
    v1 = (nh / 64).astype(f8)
    r1 = nh - 64 * v1.astype(np.float32)
    v2 = (r1 / 8).astype(f8)
    v3 = (r1 - 8 * v2.astype(np.float32)).astype(f8)
    rhsq = xq.copy()
    rhsq[:, 2045], rhsq[:, 2046], rhsq[:, 2047] = v1, v2, v3
    lhsq = xq
    lhsq[:, 2045], lhsq[:, 2046], lhsq[:, 2047] = 64, 8, 1
    rhsT = np.ascontiguousarray(rhsq.T)                      # [C, N]
    lhsT = np.ascontiguousarray(lhsq.T)
    rhs8 = np.ascontiguousarray(
        rhsT.reshape(KP, 2, 128, NJ, NB).transpose(2, 3, 0, 1, 4))
    in_maps = []
    for c in range(N_CORES):
        xrqT = lhsT[:, c * ROWS:(c + 1) * ROWS]              # [C, 784]
        lhsT8 = np.ascontiguousarray(
            xrqT.reshape(KP, 2, 128, MT, MP).transpose(2, 3, 0, 1, 4)
        ).reshape(128, MT * KP, 2, MP)
        in_maps.append({"rhs8": rhs8, "lhsT8": lhsT8})
    res = run_bass_kernel_spmd(
        _PROG, in_maps, list(range(N_CORES)),
        trace=bool(os.environ.get("KNN_TRACE")),
    )
    if res.exec_time_ns is not None:
        LAST_EXEC_NS = res.exec_time_ns

    # per-core outputs are [MT, MP, NJ, 8] -> [ROWS, 112]
    vals_all = np.concatenate(
        [r["vals"].reshape(ROWS, 112) for r in res.results], axis=0)
    loc = np.concatenate(
        [r["idxs"].reshape(ROWS, 112) for r in res.results],
        axis=0).astype(np.int64)
    idxs_all = loc + (np.arange(NJ, dtype=np.int64) * NB).repeat(8)[None, :]

    # coarse top-32 by device (fp8 matmul) score, then exact fp32 re-score
    part = np.argpartition(-vals_all, 32, axis=1)[:, :32]
    idxs_all = np.take_along_axis(idxs_all, part, axis=1)    # [N, 32]
    sq = np.sum(x_flat * x_flat, axis=1, dtype=np.float32)
    exact = np.empty((N, 32), dtype=np.float32)
    BLK = 196
    for r0 in range(0, N, BLK):
        r1 = r0 + BLK
        cand = idxs_all[r0:r1]                               # [b, 32]
        xc = x_flat[cand]                                    # [b, 32, C]
        exact[r0:r1] = np.einsum("bc,bkc->bk", x_flat[r0:r1], xc,
                                 dtype=np.float32) - 0.5 * sq[cand]
    order = np.argsort(-exact, axis=1, kind="stable")[:, :K + 1]
    top = np.take_along_axis(idxs_all, order, axis=1)        # [N, 9]
    rows = np.arange(N)[:, None]
    selfpos = top == rows
    has_self = selfpos.any(axis=1)
    rem = np.where(has_self, selfpos.argmax(axis=1), K)      # drop self, else 9th
    keep = np.ones((N, K + 1), dtype=bool)
    keep[np.arange(N), rem] = False
    global LAST_KNN
    LAST_KNN = top[keep].reshape(N, K)
    return LAST_KNN


def kernel(x, W1, b1, W2, b2):
    x = np.asarray(x, dtype=np.float32)
    W1 = np.asarray(W1, dtype=np.float32)
    b1 = np.asarray(b1, dtype=np.float32)
    W2 = np.asarray(W2, dtype=np.float32)
    b2 = np.asarray(b2, dtype=np.float32)

    xf = x.reshape(N, C)
    knn = _knn_from_device(xf)

    src = np.repeat(np.arange(N, dtype=np.int64), K)
    dst = knn.reshape(-1)
    loops = np.arange(N, dtype=np.int64)
    src = np.concatenate([src, loops])
    dst = np.concatenate([dst, loops])

    deg = np.bincount(dst, minlength=N).astype(np.float32)
    dinv = 1.0 / np.sqrt(np.maximum(deg, 1.0))
    norm = (dinv[src] * dinv[dst]).astype(np.float32)

    try:
        import scipy.sparse as sps
        A = sps.csr_matrix((norm, (dst, src)), shape=(N, N), dtype=np.float32)

        def agg(hw):
            return A @ hw
    except Exception:
        def agg(hw):
            out = np.zeros_like(hw)
            np.add.at(out, dst, hw[src] * norm[:, None])
            return out

    h1 = np.maximum(agg(xf @ W1) + b1, 0.0).astype(np.float32)
    h2 = np.maximum(agg(h1 @ W2) + b2, 0.0).astype(np.float32)
    return h2.reshape(B, H, W, W2.shape[1]).astype(np.float32)
